# revision 1
# baseline (speedup 1.0000x reference)
"""Trainium2 Bass kernel for nn_DilationLayerExtSE (morphological dilation,
external structuring element, per-sample/per-channel weights).

    out[b,c,i,j] = max_{di,dj} (xpad[b,c,i+di,j+dj] + weight[b,c,di,dj]) + bias[b,c]

Shapes (hardcoded): x (8,128,128,128) f32, weight (8,128,5,5) f32,
bias (8,128) f32, padding=2, stride=1 -> out (8,128,128,128) f32.

Sharding: data-parallel over B across the 8 NeuronCores (1 sample/core).
Per core: C=128 maps onto the 128 SBUF partitions; each channel's padded
132x132 plane is a flat 17424-element stream in that partition.  The bias is
folded into the 25 SE weights up front (max_k(p+w_k)+b == max_k(p+(w_k+b))).

Contiguous-stream trick: for a band of `rows` output rows starting at r0,
the accumulator holds L = rows*132 elements where acc[t] with t = i*132 + j
(j < 128) is out[r0+i, j].  For SE offset (di,dj) the input is the fully
contiguous slice xflat[(r0+di)*132 + dj : ... + L] — every DVE/ACT/GPSIMD
pass streams one unit-stride run (no per-row AP breaks).  Positions with
j in [128,132) compute wrapped garbage and are never stored.

Row-bands are statically assigned to lanes:
  v<rows>: DVE runs the fused scalar_tensor_tensor chain
           acc = (x_shift + w_k) max acc.
  g<rows>: ACT produces tmp = x_shift + w_k (activation-identity with
           per-partition bias), GPSIMD runs tensor_max into acc.
           (rejected by this walrus build's ISA engine check — kept for reference)
  d<rows>: ACT produces tmp, then a SWDGE DMA with accum_op=max folds it
           into acc through the SDMA CCE units.
           (rejected by this walrus build: "DMACopy does not support max
           with Copy mode" — kept for reference)
All lanes are seeded by ACT (k=0 is a pure biased copy).  With both
auxiliary lanes rejected by the toolchain the kernel is DVE-bound:
24 fused fp32 passes at 1 elem/cycle/lane, ~425 us/core.
"""

import os
import time

import numpy as np

B, C, H, W = 8, 128, 128, 128
KH = KW = 5
PAD = 2
HP, WP = H + 2 * PAD, W + 2 * PAD  # 132, 132
NK = KH * KW
XLEN = HP * WP + 4  # flat padded plane + tail so the last band's k=24 slice is in-bounds

# Band sizes are asymmetric on purpose: a small first band lets DVE start
# ~4 us after launch (only a 0.5 MB input DMA + seed ahead of it), and a
# small last band leaves only a ~0.5 MB output DMA after the final pass.
# Middle bands are large so the per-pass fixed cost stays amortized.
LANES = os.environ.get("KERNEL_LANES", "v8,v56,v56,v8")
NITER = int(os.environ.get("KERNEL_NITER", "0"))

_CACHE: dict = {}

LAST_RUN_SECONDS: float | None = None
LAST_EXEC_TIME_NS: int | None = None


def _parse_lanes():
    bands = []
    r0 = 0
    for part in LANES.split(","):
        eng, rows = part[0], int(part[1:])
        assert eng in "vgd"
        bands.append((eng, r0, rows))
        r0 += rows
    assert r0 == H, f"lanes must cover {H} rows, got {r0}"
    return bands


def _build_program():
    from contextlib import ExitStack

    import concourse.bacc as bacc
    import concourse.tile as tile
    from concourse import mybir

    bands = _parse_lanes()

    nc = bacc.Bacc("TRN2", target_bir_lowering=False, debug=False)
    dt = mybir.dt.float32
    x = nc.dram_tensor("x", [C, H, W], dt, kind="ExternalInput")
    w = nc.dram_tensor("w", [C, NK], dt, kind="ExternalInput")
    b = nc.dram_tensor("bias", [C, 1], dt, kind="ExternalInput")
    out = nc.dram_tensor("out", [C, H, W], dt, kind="ExternalOutput")

    add = mybir.AluOpType.add
    mx = mybir.AluOpType.max
    ident = mybir.ActivationFunctionType.Identity

    with tile.TileContext(nc) as tc, ExitStack() as ctx:
        const = ctx.enter_context(tc.tile_pool(name="const", bufs=1))
        accv_p = ctx.enter_context(tc.tile_pool(name="accv", bufs=2))
        accg_p = ctx.enter_context(tc.tile_pool(name="accg", bufs=2))
        tmp_p = ctx.enter_context(tc.tile_pool(name="tmp", bufs=2))

        xpad = const.tile([C, XLEN], dt)
        wraw = const.tile([C, NK], dt)
        bt = const.tile([C, 1], dt)
        wb = const.tile([C, NK], dt)

        xp3 = xpad[:, 0 : HP * WP].rearrange("c (h w) -> c h w", w=WP)
        # zero the pad borders + tail (interior is overwritten by the DMA)
        # on gpsimd (walrus accepts Pool memset): keeps DVE's startup clear
        nc.gpsimd.memset(xpad[:, 0 : PAD * WP], 0.0)
        nc.gpsimd.memset(xpad[:, (HP - PAD) * WP : XLEN], 0.0)
        nc.gpsimd.memset(xp3[:, PAD : HP - PAD, 0:PAD], 0.0)
        nc.gpsimd.memset(xp3[:, PAD : HP - PAD, WP - PAD : WP], 0.0)

        nc.sync.dma_start(out=wraw[:], in_=w[:, :])
        nc.sync.dma_start(out=bt[:], in_=b[:, :])
        # fold bias into the SE: wb = w + bias (per-partition scalar add).
        # On ACT, not DVE: the band-0 seed needs wb, and an idle-DVE wait on
        # the tiny w/bias DMAs here delays the first compute pass by ~3 us.
        nc.scalar.activation(wb[:], wraw[:], ident, bias=bt[:], scale=1.0)

        def body(_iv=None):
            # load x per band so the first band's compute starts as soon as
            # its rows land (pieces queue FIFO on the HWDGE ring)
            for _, r0, rows in bands:
                nc.sync.dma_start(
                    out=xp3[:, PAD + r0 : PAD + r0 + rows, PAD : PAD + W],
                    in_=x[:, r0 : r0 + rows, :],
                )
            for eng_c, r0, rows in bands:
                L = rows * WP
                pool = accv_p if eng_c == "v" else accg_p
                acc = pool.tile([C, L], dt, tag=f"acc_{eng_c}")

                def win(k):
                    di, dj = divmod(k, KW)
                    base = (r0 + di) * WP + dj
                    return xpad[:, base : base + L]

                # k = 0 seeds the accumulator on ACT: acc = x_win + wb[0]
                nc.scalar.activation(acc[:], win(0), ident, bias=wb[:, 0:1], scale=1.0)
                if eng_c == "v":
                    for k in range(1, NK):
                        nc.vector.scalar_tensor_tensor(
                            out=acc[:], in0=win(k), scalar=wb[:, k : k + 1],
                            in1=acc[:], op0=add, op1=mx,
                        )
                elif eng_c == "g":
                    for k in range(1, NK):
                        tmp = tmp_p.tile([C, L], dt, tag="tmp")
                        nc.scalar.activation(
                            tmp[:], win(k), ident, bias=wb[:, k : k + 1], scale=1.0
                        )
                        nc.gpsimd.tensor_max(acc[:], acc[:], tmp[:])
                else:  # "d": ACT add + SWDGE CCE accumulate-max
                    for k in range(1, NK):
                        tmp = tmp_p.tile([C, L], dt, tag="tmp")
                        nc.scalar.activation(
                            tmp[:], win(k), ident, bias=wb[:, k : k + 1], scale=1.0
                        )
                        nc.gpsimd.dma_start(out=acc[:], in_=tmp[:], accum_op=mx)
                acc3 = acc.rearrange("c (h w) -> c h w", w=WP)
                nc.sync.dma_start(out=out[:, r0 : r0 + rows, :], in_=acc3[:, :, 0:W])

        if NITER > 0:
            with tc.For_i(0, NITER, 1):
                body()
        else:
            body()

    nc.compile()
    return nc


def _get_nc():
    if "nc" not in _CACHE:
        _CACHE["nc"] = _build_program()
    return _CACHE["nc"]


def kernel(x, weight, bias, padding, stride):
    global LAST_RUN_SECONDS, LAST_EXEC_TIME_NS
    from concourse.bass_utils import run_bass_kernel_spmd

    x = np.asarray(x, dtype=np.float32)
    weight = np.asarray(weight, dtype=np.float32)
    bias = np.asarray(bias, dtype=np.float32)
    assert int(padding) == PAD and int(stride) == 1
    assert x.shape == (B, C, H, W) and weight.shape == (B, C, KH, KW)

    nc = _get_nc()
    in_maps = [
        {
            "x": np.ascontiguousarray(x[i]),
            "w": np.ascontiguousarray(weight[i].reshape(C, NK)),
            "bias": np.ascontiguousarray(bias[i].reshape(C, 1)),
        }
        for i in range(B)
    ]
    t0 = time.perf_counter()
    res = run_bass_kernel_spmd(nc, in_maps, core_ids=list(range(B)))
    LAST_RUN_SECONDS = time.perf_counter() - t0
    LAST_EXEC_TIME_NS = res.exec_time_ns
    return np.stack([res.results[i]["out"] for i in range(B)], axis=0)

